# revision 1
# baseline (speedup 1.0000x reference)
"""Trainium2 Bass kernel for nn_DAWNBlock (8-core SPMD).

Decomposition (validated against reference in numpy first):
  - Token-sharded: core c owns flat tokens [512c, 512c+512) of [B*S=4096].
    LN1, feature einsums, restore einsums (Q/K transposed layout, V
    token-major), LN2 + knowledge block all run token-locally.
  - Head-sharded attention: core c owns heads {2c, 2c+1} = d-cols
    [128c, 128c+128). AllToAll reshards QT/KT/V from token-shards to
    head-shards (2MB each, vs 16MB for an AllGather).
  - Causal softmax without max-subtraction (scores ~1e-3 magnitude, exp is
    safe); denominator via a ones-column appended to V so the PV matmul
    accumulates sum(exp) in PSUM row 64; normalization via PE-broadcast
    reciprocal.
  - W_O applied head-locally -> per-core contribution [4096, 1024];
    ReduceScatter sums + scatters token-shards back for stage C.
  - All matmuls in fp32r (full PE rate, ~1.6e-4 matmul rel err).
"""
import sys

sys.path.insert(0, "/opt/trn_rl_repo")

import numpy as np
import concourse.bass as bass
import concourse.mybir as mybir
import concourse.tile as tile
from concourse import bacc
from concourse.bass_utils import run_bass_kernel_spmd
from concourse.masks import make_identity

B, S, D, H, R, N, KR = 2, 2048, 1024, 16, 256, 8, 128
DH = D // H
BS = B * S
NC = 8
T = BS // NC          # 512 tokens per core
P = 128
TT = T // P           # 4 token tiles per core
DC = D // P           # 8 d chunks
RC = R // P           # 2 r chunks
EPS = 1e-5

import os
STAGES = int(os.environ.get("BASS_STAGES", "5"))
REPS = int(os.environ.get("BASS_REPS", "1"))
F32 = mybir.dt.float32
F32R = mybir.dt.float32r
AF = mybir.ActivationFunctionType
OP = mybir.AluOpType


def _layernorm(nc, ctx, tc, cpool, x_sb, nx_sb, gbc, bbc, eps_tile, tag):
    """nx = (x - mean(x)) * rsqrt(var + eps) * g + b for one [128, D] tile."""
    s = cpool.tile([P, 1], F32, tag="ln_s", name=f"{tag}_s")
    nm = cpool.tile([P, 1], F32, tag="ln_nm", name=f"{tag}_nm")
    sq = cpool.tile([P, D], F32, tag="ln_sq", name=f"{tag}_sq")
    ssq = cpool.tile([P, 1], F32, tag="ln_ssq", name=f"{tag}_ssq")
    sd = cpool.tile([P, 1], F32, tag="ln_sd", name=f"{tag}_sd")
    rs = cpool.tile([P, 1], F32, tag="ln_rs", name=f"{tag}_rs")
    nmrs = cpool.tile([P, 1], F32, tag="ln_nmrs", name=f"{tag}_nmrs")
    tmp = cpool.tile([P, D], F32, tag="ln_tmp", name=f"{tag}_tmp")
    nc.vector.reduce_sum(s[:], x_sb[:], axis=mybir.AxisListType.X)
    nc.vector.tensor_scalar_mul(nm[:], s[:], -1.0 / D)
    # sq = (x - mean)^2, ssq = row sum of sq
    nc.scalar.activation(sq[:], x_sb[:], AF.Square, bias=nm[:], accum_out=ssq[:])
    # sd = sqrt(ssq/D + eps)
    nc.scalar.activation(sd[:], ssq[:], AF.Sqrt, bias=eps_tile[:], scale=1.0 / D)
    nc.vector.reciprocal(rs[:], sd[:])
    nc.vector.tensor_mul(nmrs[:], nm[:], rs[:])
    # tmp = (x - mean) * rs ; nx = tmp * g + b
    nc.scalar.activation(tmp[:], x_sb[:], AF.Identity, bias=nmrs[:], scale=rs[:])
    nc.vector.tensor_mul(tmp[:], tmp[:], gbc[:])
    nc.vector.tensor_add(nx_sb[:], tmp[:], bbc[:])


def _build():
    nc = bacc.Bacc("TRN2", target_bir_lowering=False, debug=False, num_devices=NC)

    di = lambda name, shape: nc.dram_tensor(name, shape, F32, kind="ExternalInput").ap()
    x_in = di("x_sh", [T, D])
    w_in = {k: di(k, [T, N]) for k in
            ["wfq", "wfk", "wfv", "wrq", "wrk", "wrv", "wkf", "wkr"]}
    fqk_in = di("fqk_p", [N, D, R])
    fv_in = di("fv_p", [N, D, R])
    rqk_in = di("rqk_p", [N, R, D])
    rv_in = di("rv_p", [N, R, D])
    fkn_in = di("fkn_p", [N, D, KR])
    rkn_in = di("rkn_p", [N, KR, D])
    wo_in = di("wo_my", [P, D])
    masku_in = di("masku", [P, P])
    ln_in = {k: di(k, [1, D]) for k in ["ln1g", "ln1b", "ln2g", "ln2b"]}
    out_ap = nc.dram_tensor("out_sh", [T, D], F32, kind="ExternalOutput").ap()

    with tile.TileContext(nc) as tc:
        from contextlib import ExitStack
        with ExitStack() as ctx:
            # ---------- persistent pools ----------
            const = ctx.enter_context(tc.tile_pool(name="const", bufs=1))
            cpool = ctx.enter_context(tc.tile_pool(name="scratch", bufs=2))
            dram = ctx.enter_context(tc.tile_pool(name="dram", bufs=1, space="DRAM"))
            pers = ctx.enter_context(tc.tile_pool(name="pers", bufs=1))

            ident_f = const.tile([P, P], F32, tag="ident_f", name="ident_f")
            make_identity(nc, ident_f)
            ident = const.tile([P, P], F32R, tag="ident", name="ident")
            nc.vector.tensor_copy(ident[:], ident_f[:])
            masku = const.tile([P, P], F32R, tag="masku", name="masku")
            nc.sync.dma_start(masku[:], masku_in[:].bitcast(F32R))
            ones1_f = const.tile([1, P], F32, tag="ones1_f", name="ones1_f")
            nc.vector.memset(ones1_f[:], 1.0)
            ones1 = const.tile([1, P], F32R, tag="ones1", name="ones1")
            nc.vector.tensor_copy(ones1[:], ones1_f[:])
            onescol_f = const.tile([P, S // P, 1], F32, tag="onescol_f", name="onescol_f")
            nc.vector.memset(onescol_f[:], 1.0)
            zeros_f = const.tile([P, 384], F32, tag="zeros_f", name="zeros_f")
            nc.vector.memset(zeros_f[:], 0.0)
            eps_t = const.tile([P, 1], F32, tag="eps", name="eps")
            nc.vector.memset(eps_t[:], EPS)

            # broadcast ln params to [128, D] via PE (lhsT=ones [1,128], rhs=g [1,512])
            lnbc = {}
            with tc.tile_pool(name="ps_init", bufs=2, space="PSUM") as ps_init, \
                 tc.tile_pool(name="lnrow", bufs=1) as lnrow:
                for k in ["ln1g", "ln1b", "ln2g", "ln2b"]:
                    row = lnrow.tile([1, D], F32R, tag=f"{k}_row", name=f"{k}_row")
                    nc.sync.dma_start(row[:], ln_in[k][:].bitcast(F32R))
                    bc = const.tile([P, D], F32, tag=f"{k}_bc", name=f"{k}_bc")
                    for jf in range(2):
                        psb = ps_init.tile([P, 512], F32, tag="lnbc", name="lnbc")
                        nc.tensor.matmul(psb[:], ones1[:], row[:, 512 * jf:512 * (jf + 1)])
                        nc.vector.tensor_copy(bc[:, 512 * jf:512 * (jf + 1)], psb[:])
                    lnbc[k] = bc

            # routing weight columns, per token-tile
            wsb = {}
            for k in w_in:
                wt = const.tile([P, TT, N], F32, tag=f"w_{k}", name=f"w_{k}")
                nc.sync.dma_start(wt[:], w_in[k].rearrange("(tt p) n -> p tt n", p=P))
                wsb[k] = wt

            for _rep in range(REPS):
                # ---------- DRAM bounce buffers ----------
                qt_b = dram.tile([D, T], F32, tag="qt_b", name="qt_b")
                kt_b = dram.tile([D, T], F32, tag="kt_b", name="kt_b")
                v_b = dram.tile([BS, P], F32, tag="v_b", name="v_b")
                qt_o = dram.tile([D, T], F32, tag="qt_o", name="qt_o")
                kt_o = dram.tile([D, T], F32, tag="kt_o", name="kt_o")
                v_o = dram.tile([BS, P], F32, tag="v_o", name="v_o")
                wo_b = dram.tile([BS, D], F32, tag="wo_b", name="wo_b")
                ao_b = dram.tile([T, D], F32, tag="ao_b", name="ao_b")

                # ---------- stage A: LN1 + transposes + features ----------
                h_q = [pers.tile([P, R], F32, tag=f"hq{tt}", name=f"hq{tt}") for tt in range(TT)]
                h_k = [pers.tile([P, R], F32, tag=f"hk{tt}", name=f"hk{tt}") for tt in range(TT)]
                h_v = [pers.tile([P, R], F32, tag=f"hv{tt}", name=f"hv{tt}") for tt in range(TT)]

                with tc.tile_pool(name="ps_a", bufs=4, space="PSUM") as ps_a, \
                     tc.tile_pool(name="ps_at", bufs=2, space="PSUM") as ps_at, \
                     tc.tile_pool(name="fstr", bufs=2) as fstr, \
                     tc.tile_pool(name="stagea", bufs=1) as stagea:
                    nxT = [stagea.tile([P, T], F32R, tag=f"nxT{dc}", name=f"nxT{dc}") for dc in range(DC)]
                    for tt in range(TT):
                        x_t = stagea.tile([P, D], F32, tag="xa", name=f"xa{tt}", bufs=2)
                        nx_t = stagea.tile([P, D], F32R, tag="nxa", name=f"nxa{tt}", bufs=2)
                        nc.sync.dma_start(x_t[:], x_in[P * tt:P * (tt + 1), :])
                        _layernorm(nc, ctx, tc, cpool, x_t, nx_t,
                                   lnbc["ln1g"], lnbc["ln1b"], eps_t, f"ln1_{tt}")
                        for dc in range(DC):
                            pst = ps_at.tile([P, P], F32R, tag="tp", name="tp")
                            nc.tensor.transpose(pst[:], nx_t[:, P * dc:P * (dc + 1)], ident[:])
                            nc.vector.tensor_copy(nxT[dc][:, P * tt:P * (tt + 1)], pst[:])

                    for src_ap, hs, wkey, wkey2 in [
                        (fqk_in, (h_q, h_k), "wfq", "wfk"),
                        (fv_in, (h_v,), "wfv", None),
                    ]:
                        for m in range(N):
                            ft = fstr.tile([P, DC, R], F32R, tag="fstr", name="fstr")
                            nc.sync.dma_start(
                                ft[:], src_ap[m].rearrange("(dc p) r -> p dc r", p=P).bitcast(F32R))
                            for tt in range(TT):
                                ps = ps_a.tile([P, R], F32, tag="pm", name="pm")
                                for dc in range(DC):
                                    nc.tensor.matmul(
                                        ps[:], nxT[dc][:, P * tt:P * (tt + 1)], ft[:, dc, :],
                                        start=(dc == 0), stop=(dc == DC - 1))
                                targets = [(hs[0], wkey)] + ([(hs[1], wkey2)] if wkey2 else [])
                                for htiles, wk in targets:
                                    wcol = wsb[wk][:, tt, m:m + 1]
                                    if m == 0:
                                        nc.vector.tensor_scalar_mul(htiles[tt][:], ps[:], wcol)
                                    else:
                                        nc.vector.scalar_tensor_tensor(
                                            htiles[tt][:], ps[:], wcol, htiles[tt][:],
                                            op0=OP.mult, op1=OP.add)

                if STAGES == 1:
                    for tt in range(TT):
                        hq_ev = cpool.tile([P, R], F32, tag="hq_ev", name=f"hq_ev{tt}")
                        nc.vector.tensor_copy(hq_ev[:], h_q[tt][:])
                        nc.sync.dma_start(out_ap[P * tt:P * (tt + 1), 0:R], hq_ev[:])
                # ---------- stage B: restores (Q, K with r_qk; V with r_v) ----------
                with tc.tile_pool(name="rp", bufs=1) as rp, \
                     tc.tile_pool(name="ap_pool", bufs=1) as ap_pool, \
                     tc.tile_pool(name="ps_b", bufs=4, space="PSUM") as ps_b, \
                     tc.tile_pool(name="ps_bt", bufs=2, space="PSUM") as ps_bt, \
                     tc.tile_pool(name="ev_b", bufs=2) as ev_b:
                    r_sb = rp.tile([P, N, RC, D], F32R, tag="rqkv", name="rqkv")
                    nc.sync.dma_start(
                        r_sb[:], rqk_in.rearrange("n (rc p) d -> p n rc d", p=P).bitcast(F32R))

                    A = [ap_pool.tile([P, RC, T], F32R, tag=f"A{n}", name=f"A{n}") for n in range(N)]

                    def build_A(h_tiles, wkey):
                        for n in range(N):
                            for tt in range(TT):
                                a_tok = cpool.tile([P, R], F32R, tag="a_tok", name="a_tok")
                                nc.scalar.activation(a_tok[:], h_tiles[tt][:], AF.Copy,
                                                     scale=wsb[wkey][:, tt, n:n + 1])
                                for rc in range(RC):
                                    pst = ps_bt.tile([P, P], F32R, tag="tpb", name="tpb")
                                    nc.tensor.transpose(pst[:], a_tok[:, P * rc:P * (rc + 1)], ident[:])
                                    nc.vector.tensor_copy(A[n][:, rc, P * tt:P * (tt + 1)], pst[:])

                    def qk_restore(dst_dram):
                        for dm in range(DC):
                            ps = ps_b.tile([P, T], F32, tag="qk_ps", name="qk_ps")
                            first = True
                            for n in range(N):
                                for rc in range(RC):
                                    nc.tensor.matmul(
                                        ps[:], r_sb[:, n, rc, P * dm:P * (dm + 1)], A[n][:, rc, :],
                                        start=first, stop=(n == N - 1 and rc == RC - 1))
                                    first = False
                            ev = ev_b.tile([P, T], F32, tag="ev_qk", name="ev_qk")
                            nc.vector.tensor_copy(ev[:], ps[:])
                            nc.sync.dma_start(dst_dram[P * dm:P * (dm + 1), :], ev[:])

                    build_A(h_q, "wrq")
                    qk_restore(qt_b)
                    build_A(h_k, "wrk")
                    qk_restore(kt_b)

                    nc.gpsimd.collective_compute(
                        "AllToAll", OP.bypass, replica_groups=[list(range(NC))],
                        ins=[qt_b.opt()], outs=[qt_o.opt()])
                    nc.gpsimd.collective_compute(
                        "AllToAll", OP.bypass, replica_groups=[list(range(NC))],
                        ins=[kt_b.opt()], outs=[kt_o.opt()])

                    rv_sb = rp.tile([P, N, RC, D], F32R, tag="rqkv", name="rqkv")
                    nc.sync.dma_start(
                        rv_sb[:], rv_in.rearrange("n (rc p) d -> p n rc d", p=P).bitcast(F32R))
                    build_A(h_v, "wrv")
                    for tt in range(TT):
                        for jf in range(2):
                            ps = ps_b.tile([P, 512], F32, tag="qk_ps", name="v_ps")
                            first = True
                            for n in range(N):
                                for rc in range(RC):
                                    nc.tensor.matmul(
                                        ps[:], A[n][:, rc, P * tt:P * (tt + 1)],
                                        rv_sb[:, n, rc, 512 * jf:512 * (jf + 1)],
                                        start=first, stop=(n == N - 1 and rc == RC - 1))
                                    first = False
                            ev = ev_b.tile([P, 512], F32, tag="ev_v", name="ev_v")
                            nc.vector.tensor_copy(ev[:], ps[:])
                            for db in range(4):
                                d = 4 * jf + db
                                nc.sync.dma_start(
                                    v_b[T * d + P * tt: T * d + P * (tt + 1), :],
                                    ev[:, P * db:P * (db + 1)])
                    nc.gpsimd.collective_compute(
                        "AllToAll", OP.bypass, replica_groups=[list(range(NC))],
                        ins=[v_b.opt()], outs=[v_o.opt()])

                if STAGES == 2:
                    for tt in range(TT):
                        qo_ev = cpool.tile([P, T], F32, tag="qo_ev", name=f"qo_ev{tt}")
                        nc.sync.dma_start(qo_ev[:], qt_o[P * tt:P * (tt + 1), :])
                        nc.sync.dma_start(out_ap[P * tt:P * (tt + 1), 0:T], qo_ev[:])
                if STAGES >= 3:  # ---------- attention (head-sharded) ----------
                    attn_ctx = ctx.enter_context(ExitStack())
                    attn_pool = attn_ctx.enter_context(tc.tile_pool(name="attn_pers", bufs=1))
                    attnT = attn_pool.tile([P, BS], F32R, tag="attnT", name="attnT")
                    with tc.tile_pool(name="qkv_bh", bufs=2) as qkv_bh, \
                         tc.tile_pool(name="pt_pool", bufs=4) as pt_pool, \
                         tc.tile_pool(name="ps_st", bufs=3, space="PSUM") as ps_st, \
                         tc.tile_pool(name="ps_o", bufs=2, space="PSUM") as ps_o, \
                         tc.tile_pool(name="ps_bc", bufs=2, space="PSUM") as ps_bc:
                        for b in range(B):
                            for h2 in range(2):
                                qt_t = qkv_bh.tile([DH, S], F32R, tag="qt_bh", name="qt_bh")
                                kt_t = qkv_bh.tile([DH, S], F32R, tag="kt_bh", name="kt_bh")
                                vp = qkv_bh.tile([P, S // P, DH + 1], F32R, tag="v_bh", name="v_bh")
                                for sl in range(4):
                                    s = 4 * b + sl
                                    nc.sync.dma_start(
                                        qt_t[:, 512 * sl:512 * (sl + 1)],
                                        qt_o[P * s + DH * h2: P * s + DH * (h2 + 1), :].bitcast(F32R))
                                    nc.sync.dma_start(
                                        kt_t[:, 512 * sl:512 * (sl + 1)],
                                        kt_o[P * s + DH * h2: P * s + DH * (h2 + 1), :].bitcast(F32R))
                                nc.sync.dma_start(
                                    vp[:, :, 0:DH],
                                    v_o[S * b: S * (b + 1), DH * h2: DH * (h2 + 1)]
                                    .rearrange("(kt p) f -> p kt f", p=P).bitcast(F32R))
                                nc.vector.tensor_copy(vp[:, :, DH:DH + 1], onescol_f[:])

                                for qg in range(4):
                                    o_ps = ps_o.tile([DH + 1, 512], F32, tag="o_ps", name="o_ps")
                                    nkt = 4 * qg + 4
                                    for kt in range(nkt):
                                        st = ps_st.tile([P, 512], F32, tag="st", name="st")
                                        nc.tensor.matmul(
                                            st[:], kt_t[:, P * kt:P * (kt + 1)],
                                            qt_t[:, 512 * qg:512 * (qg + 1)])
                                        pt = pt_pool.tile([P, 512], F32R, tag="pt", name="pt")
                                        j = kt - 4 * qg
                                        if j < 0:
                                            nc.scalar.activation(pt[:], st[:], AF.Exp, scale=0.125)
                                        else:
                                            if j > 0:
                                                nc.vector.tensor_copy(pt[:, 0:P * j], zeros_f[:, 0:P * j])
                                            nc.scalar.activation(pt[:, P * j:], st[:, P * j:],
                                                                 AF.Exp, scale=0.125)
                                            nc.vector.tensor_mul(pt[:, P * j:P * (j + 1)],
                                                                 pt[:, P * j:P * (j + 1)], masku[:])
                                        nc.tensor.matmul(o_ps[:], vp[:, kt, :], pt[:],
                                                         start=(kt == 0), stop=(kt == nkt - 1))
                                    den = cpool.tile([1, 512], F32, tag="den", name="den")
                                    nc.vector.tensor_copy(den[:], o_ps[DH:DH + 1, :])
                                    rec = cpool.tile([1, 512], F32R, tag="rec", name="rec")
                                    with nc.allow_low_precision(reason="f32r rounding for PE broadcast"):
                                        nc.vector.reciprocal(rec[:], den[:])
                                    bc = ps_bc.tile([DH, 512], F32, tag="bc", name="bc")
                                    nc.tensor.matmul(bc[:], ones1[:, 0:DH], rec[:])
                                    bc_sb = cpool.tile([DH, 512], F32, tag="bc_sb", name="bc_sb")
                                    nc.scalar.activation(bc_sb[:], bc[:], AF.Copy)
                                    nc.vector.tensor_mul(
                                        attnT[DH * h2:DH * (h2 + 1),
                                              S * b + 512 * qg: S * b + 512 * (qg + 1)],
                                        o_ps[0:DH, :], bc_sb[:])

                    if STAGES == 3:
                        for tt in range(TT):
                            at_ev = cpool.tile([P, D], F32, tag="at_ev", name=f"at_ev{tt}")
                            nc.vector.tensor_copy(at_ev[:], attnT[:, D * tt:D * (tt + 1)].bitcast(F32))
                            nc.sync.dma_start(out_ap[P * tt:P * (tt + 1), :], at_ev[:])
                    if STAGES >= 4:  # ---------- W_O contribution + ReduceScatter ----------
                        with tc.tile_pool(name="ps_wo", bufs=4, space="PSUM") as ps_wo, \
                             tc.tile_pool(name="ev_wo", bufs=4) as ev_wo:
                            wo_sb = attn_pool.tile([P, D], F32R, tag="wo_sb", name="wo_sb")
                            nc.sync.dma_start(wo_sb[:], wo_in[:].bitcast(F32R))
                            for tt in range(BS // P):
                                for jf in range(2):
                                    ps = ps_wo.tile([P, 512], F32, tag="wo_ps", name="wo_ps")
                                    nc.tensor.matmul(ps[:], attnT[:, P * tt:P * (tt + 1)],
                                                     wo_sb[:, 512 * jf:512 * (jf + 1)])
                                    ev = ev_wo.tile([P, 512], F32, tag="ev_wo", name="ev_wo")
                                    nc.any.tensor_copy(ev[:], ps[:])
                                    nc.sync.dma_start(
                                        wo_b[P * tt:P * (tt + 1), 512 * jf:512 * (jf + 1)], ev[:])
                            nc.gpsimd.collective_compute(
                                "ReduceScatter", OP.add, replica_groups=[list(range(NC))],
                                ins=[wo_b.opt()], outs=[ao_b.opt()])
                        attn_ctx.close()

                        if STAGES == 4:
                            for tt in range(TT):
                                ao_ev = cpool.tile([P, D], F32, tag="ao_ev", name=f"ao_ev{tt}")
                                nc.sync.dma_start(ao_ev[:], ao_b[P * tt:P * (tt + 1), :])
                                nc.sync.dma_start(out_ap[P * tt:P * (tt + 1), :], ao_ev[:])
                        if STAGES >= 5:  # ---------- stage C: residual + LN2 + knowledge ----------
                            with tc.tile_pool(name="ps_c", bufs=3, space="PSUM") as ps_c, \
                                 tc.tile_pool(name="ps_ct", bufs=2, space="PSUM") as ps_ct, \
                                 tc.tile_pool(name="kstr", bufs=2) as kstr, \
                                 tc.tile_pool(name="cscr", bufs=2) as cscr, \
                                 tc.tile_pool(name="cper", bufs=1) as cper:
                                x2 = [cper.tile([P, D], F32, tag=f"x2_{tt}", name=f"x2_{tt}") for tt in range(TT)]
                                nx2 = [cper.tile([P, D], F32R, tag="nx2", name=f"nx2_{tt}", bufs=2) for tt in range(TT)]
                                nx2T = [cper.tile([P, T], F32R, tag=f"nx2T{dc}", name=f"nx2T{dc}") for dc in range(DC)]
                                hkn = [cper.tile([P, KR], F32, tag=f"hkn{tt}", name=f"hkn{tt}") for tt in range(TT)]
                                Akn = [cper.tile([P, T], F32R, tag=f"Akn{n}", name=f"Akn{n}") for n in range(N)]

                                for tt in range(TT):
                                    ao_sb = cscr.tile([P, D], F32, tag="ao_sb", name="ao_sb")
                                    nc.sync.dma_start(ao_sb[:], ao_b[P * tt:P * (tt + 1), :])
                                    xc = cscr.tile([P, D], F32, tag="xc", name=f"xc{tt}")
                                    nc.sync.dma_start(xc[:], x_in[P * tt:P * (tt + 1), :])
                                    nc.vector.tensor_add(x2[tt][:], xc[:], ao_sb[:])
                                    _layernorm(nc, ctx, tc, cpool, x2[tt], nx2[tt],
                                               lnbc["ln2g"], lnbc["ln2b"], eps_t, f"ln2_{tt}")
                                    for dc in range(DC):
                                        pst = ps_ct.tile([P, P], F32R, tag="tpc", name="tpc")
                                        nc.tensor.transpose(pst[:], nx2[tt][:, P * dc:P * (dc + 1)], ident[:])
                                        nc.vector.tensor_copy(nx2T[dc][:, P * tt:P * (tt + 1)], pst[:])

                                # knowledge feature: two experts per matmul ([*, 256] free)
                                for mp in range(N // 2):
                                    fk = kstr.tile([P, DC, 2 * KR], F32R, tag="fkn", name="fkn")
                                    for half in range(2):
                                        m = 2 * mp + half
                                        nc.sync.dma_start(
                                            fk[:, :, KR * half:KR * (half + 1)],
                                            fkn_in[m].rearrange("(dc p) f -> p dc f", p=P).bitcast(F32R))
                                    for tt in range(TT):
                                        ps = ps_c.tile([P, 2 * KR], F32, tag="pkn", name="pkn", bufs=3)
                                        for dc in range(DC):
                                            nc.tensor.matmul(ps[:], nx2T[dc][:, P * tt:P * (tt + 1)],
                                                             fk[:, dc, :], start=(dc == 0), stop=(dc == DC - 1))
                                        for half in range(2):
                                            m = 2 * mp + half
                                            wcol = wsb["wkf"][:, tt, m:m + 1]
                                            pshalf = ps[:, KR * half:KR * (half + 1)]
                                            if m == 0:
                                                nc.vector.tensor_scalar_mul(hkn[tt][:], pshalf, wcol)
                                            else:
                                                nc.vector.scalar_tensor_tensor(
                                                    hkn[tt][:], pshalf, wcol, hkn[tt][:],
                                                    op0=OP.mult, op1=OP.add)

                                # knowledge restore
                                rk_sb = cper.tile([P, N, D], F32R, tag="rkn", name="rkn")
                                nc.sync.dma_start(rk_sb[:], rkn_in.rearrange("n p d -> p n d").bitcast(F32R))
                                for n in range(N):
                                    for tt in range(TT):
                                        a_tok = cscr.tile([P, KR], F32R, tag="akn_tok", name="akn_tok")
                                        nc.scalar.activation(a_tok[:], hkn[tt][:], AF.Copy,
                                                             scale=wsb["wkr"][:, tt, n:n + 1])
                                        pst = ps_ct.tile([P, P], F32R, tag="tpc", name="tpc")
                                        nc.tensor.transpose(pst[:], a_tok[:], ident[:])
                                        nc.vector.tensor_copy(Akn[n][:, P * tt:P * (tt + 1)], pst[:])
                                for tt in range(TT):
                                    for jf in range(2):
                                        ps = ps_c.tile([P, 512], F32, tag="kn_ps", name="kn_ps", bufs=3)
                                        first = True
                                        for n in range(N):
                                            nc.tensor.matmul(ps[:], Akn[n][:, P * tt:P * (tt + 1)],
                                                             rk_sb[:, n, 512 * jf:512 * (jf + 1)],
                                                             start=first, stop=(n == N - 1))
                                            first = False
                                        out_sb = cscr.tile([P, 512], F32, tag="out_sb", name="out_sb")
                                        nc.vector.tensor_add(out_sb[:], x2[tt][:, 512 * jf:512 * (jf + 1)], ps[:])
                                        nc.sync.dma_start(
                                            out_ap[P * tt:P * (tt + 1), 512 * jf:512 * (jf + 1)], out_sb[:])

    nc.compile()
    return nc


_NC = None


def _get_nc():
    global _NC
    if _NC is None:
        _NC = _build()
    return _NC


def kernel(**inputs):
    nc = _get_nc()
    inp = {k: np.ascontiguousarray(np.asarray(v, dtype=np.float32)) for k, v in inputs.items()}
    x_flat = inp["x"].reshape(BS, D)
    wf = {
        "wfq": inp["fqk_w_Q"].reshape(BS, N), "wfk": inp["fqk_w_K"].reshape(BS, N),
        "wfv": inp["fv_w"].reshape(BS, N), "wrq": inp["rqk_w_Q"].reshape(BS, N),
        "wrk": inp["rqk_w_K"].reshape(BS, N), "wrv": inp["rv_w"].reshape(BS, N),
        "wkf": inp["feature_know_w"].reshape(BS, N),
        "wkr": inp["restore_know_w"].reshape(BS, N),
    }
    W_OT = np.ascontiguousarray(inp["W_O"].T)
    masku = np.ascontiguousarray(np.tril(np.ones((P, P), np.float32)).T)

    in_maps = []
    for c in range(NC):
        sl = slice(T * c, T * (c + 1))
        m = {
            "x_sh": x_flat[sl],
            "fqk_p": inp["f_qk"], "fv_p": inp["f_v"],
            "rqk_p": inp["r_qk"], "rv_p": inp["r_v"],
            "fkn_p": inp["f_know"], "rkn_p": inp["r_know"],
            "wo_my": np.ascontiguousarray(W_OT[P * c:P * (c + 1), :]),
            "masku": masku,
            "ln1g": inp["ln1_g"].reshape(1, D), "ln1b": inp["ln1_b"].reshape(1, D),
            "ln2g": inp["ln2_g"].reshape(1, D), "ln2b": inp["ln2_b"].reshape(1, D),
        }
        for k, v in wf.items():
            m[k] = np.ascontiguousarray(v[sl])
        in_maps.append(m)

    res = run_bass_kernel_spmd(nc, in_maps, list(range(NC))).results
    out = np.concatenate([res[c]["out_sh"] for c in range(NC)], axis=0)
    return out.reshape(B, S, D)



# revision 7
# speedup vs baseline: 1.9082x; 1.9082x over previous
"""Trainium2 Bass kernel for nn_DAWNBlock (8-core SPMD), v3.

Decomposition (validated in numpy: proto_check.py, quant_check.py):
  - Token-sharded: core c owns flat tokens [512c, 512c+512) of [B*S=4096].
    LN1, feature einsums, restores (Q/K/V), LN2 + knowledge run token-local.
  - Head-sharded attention: core c owns d-cols [128c, 128c+128) (= heads
    {2c, 2c+1}). One AllToAll reshards Q^T+K^T together (1MB fp8), a second
    reshards V (0.5MB fp8); attention output is AllToAll'd back to token
    shards (0.5MB fp8) and W_O applied token-locally.
  - fp8e4m3 + DoubleRow (K=256 pairs, 2x PE rate) for features, restores,
    PV, W_O and knowledge matmuls; scores fp8 at K=64 packed as concurrent
    row-group pairs (both heads). Quantization study: all-fp8 rel err 4e-3
    vs the 2e-2 gate. Accumulation fp32 in PSUM; LN stats + residual fp32.
  - Routing weights: feature PSUM banks combined via DVE scalar_tensor_tensor;
    restores use A[n] = hT * wbc[n] (wbc = PE-broadcast row of host-
    pretransposed weights), h transposed once.
  - Causal softmax without max-subtraction; denominator via ones-column in V
    (PSUM row 64); reciprocal via ACT rsqrt + squared broadcast multiply.
"""
import sys

sys.path.insert(0, "/opt/trn_rl_repo")

import os
import numpy as np
import ml_dtypes
import concourse.bass as bass
import concourse.mybir as mybir
import concourse.tile as tile
from concourse import bacc
from concourse.bass_utils import run_bass_kernel_spmd
from concourse.masks import make_identity

B, S, D, H, R, N, KR = 2, 2048, 1024, 16, 256, 8, 128
DH = D // H           # 64
BS = B * S            # 4096
NC = 8
T = BS // NC          # 512 tokens per core
P = 128
TT = T // P           # 4 token tiles per core
DC = D // P           # 8 d chunks
DCP = DC // 2         # 4 d chunk-pairs (DoubleRow)
RC = R // P           # 2 r chunks
EPS = 1e-5

STAGES = int(os.environ.get("BASS_STAGES", "5"))
F32 = mybir.dt.float32
F32R = mybir.dt.float32r
BF = mybir.dt.bfloat16
F8 = mybir.dt.float8e4
DR = mybir.MatmulPerfMode.DoubleRow
AF = mybir.ActivationFunctionType
OP = mybir.AluOpType


def _layernorm(nc, cpool, x_sb, nx_sb, gbc, bbc, eps_tile, tag):
    """nx = (x - mean(x)) * rsqrt(var + eps) * g + b for one [128, D] tile."""
    s = cpool.tile([P, 1], F32, tag="ln_s", name=f"{tag}_s")
    nm = cpool.tile([P, 1], F32, tag="ln_nm", name=f"{tag}_nm")
    sq = cpool.tile([P, D], F32, tag="ln_sq", name=f"{tag}_sq")
    ssq = cpool.tile([P, 1], F32, tag="ln_ssq", name=f"{tag}_ssq")
    rs = cpool.tile([P, 1], F32, tag="ln_rs", name=f"{tag}_rs")
    nmrs = cpool.tile([P, 1], F32, tag="ln_nmrs", name=f"{tag}_nmrs")
    tmp = cpool.tile([P, D], F32, tag="ln_tmp", name=f"{tag}_tmp")
    nc.vector.reduce_sum(s[:], x_sb[:], axis=mybir.AxisListType.X)
    nc.vector.tensor_scalar_mul(nm[:], s[:], -1.0 / D)
    nc.scalar.activation(sq[:], x_sb[:], AF.Square, bias=nm[:], accum_out=ssq[:])
    # rs = 1/sqrt(ssq/D + eps)
    nc.scalar.activation(rs[:], ssq[:], AF.Abs_reciprocal_sqrt,
                         bias=eps_tile[:], scale=1.0 / D)
    nc.vector.tensor_mul(nmrs[:], nm[:], rs[:])
    nc.scalar.activation(tmp[:], x_sb[:], AF.Identity, bias=nmrs[:], scale=rs[:])
    nc.vector.tensor_mul(tmp[:], tmp[:], gbc[:])
    nc.vector.tensor_add(nx_sb[:], tmp[:], bbc[:])


def _build():
    nc = bacc.Bacc("TRN2", target_bir_lowering=False, debug=False, num_devices=NC)

    def di(name, shape, dt=F8):
        return nc.dram_tensor(name, shape, dt, kind="ExternalInput").ap()

    x_in = di("x_sh", [T, D], F32)
    wcol_in = {k: di(k, [T, N], F32) for k in ["wfq", "wfk", "wfv", "wkf"]}
    wrow_in = {k: di(k, [N, T], BF) for k in ["wrqT", "wrkT", "wrvT", "wkrT"]}
    fqk_in = di("fqk_p", [N, D, R])
    fv_in = di("fv_p", [N, D, R])
    rqk_in = di("rqk_p", [N, R, D])
    rv_in = di("rv_p", [N, R, D])
    fkn_in = di("fkn_p", [N, D, KR])
    rkn_in = di("rkn_p", [N, KR, D])
    wo_in = di("wo_p", [D, D])       # = W_O.T
    masku_in = di("masku", [P, P])
    ln_in = {k: di(k, [1, D], F32) for k in ["ln1g", "ln1b", "ln2g", "ln2b"]}
    out_ap = nc.dram_tensor("out_sh", [T, D], F32, kind="ExternalOutput").ap()

    with tile.TileContext(nc) as tc:
        from contextlib import ExitStack
        with ExitStack() as ctx:
            const = ctx.enter_context(tc.tile_pool(name="const", bufs=1))
            cpool = ctx.enter_context(tc.tile_pool(name="scratch", bufs=2))
            dram = ctx.enter_context(tc.tile_pool(name="dram", bufs=1, space="DRAM"))

            # ---------- persistent pools / big prefetches first ----------
            xpool = ctx.enter_context(tc.tile_pool(name="xpool", bufs=1))
            x_t = [xpool.tile([P, D], F32, tag=f"x{tt}", name=f"x{tt}") for tt in range(TT)]
            for tt in range(TT):
                nc.sync.dma_start(x_t[tt][:], x_in[P * tt:P * (tt + 1), :])

            fwpool = ctx.enter_context(tc.tile_pool(name="fwpool", bufs=1))
            fqk_sb = fwpool.tile([P, DCP, 2, N, R], F8, tag="fqk_sb", name="fqk_sb")
            fv_sb = fwpool.tile([P, DCP, 2, N, R], F8, tag="fv_sb", name="fv_sb")
            for dcp in range(DCP):
                for i in range(2):
                    dc = 2 * dcp + i
                    nc.sync.dma_start(
                        fqk_sb[:, dcp, i],
                        fqk_in[:, P * dc:P * (dc + 1), :].rearrange("n p r -> p n r"))
                    nc.sync.dma_start(
                        fv_sb[:, dcp, i],
                        fv_in[:, P * dc:P * (dc + 1), :].rearrange("n p r -> p n r"))

            ident_f = const.tile([P, P], F32, tag="ident_f", name="ident_f")
            make_identity(nc, ident_f)
            ident8 = const.tile([P, P], F8, tag="ident8", name="ident8")
            nc.vector.tensor_copy(ident8[:], ident_f[:])
            identb = const.tile([P, P], BF, tag="identb", name="identb")
            nc.vector.tensor_copy(identb[:], ident_f[:])
            ones_bf = const.tile([1, P], BF, tag="ones_bf", name="ones_bf")
            nc.vector.memset(ones_bf[:], 1.0)
            ones1_f = const.tile([1, P], F32, tag="ones1_f", name="ones1_f")
            nc.vector.memset(ones1_f[:], 1.0)
            ones1r = const.tile([1, P], F32R, tag="ones1r", name="ones1r")
            nc.vector.tensor_copy(ones1r[:], ones1_f[:])
            masku = const.tile([P, P], F8, tag="masku", name="masku")
            nc.sync.dma_start(masku[:], masku_in[:])
            eps_t = const.tile([P, 1], F32, tag="eps", name="eps")
            nc.vector.memset(eps_t[:], EPS)

            lnbc = {}
            with tc.tile_pool(name="ps_init", bufs=2, space="PSUM") as ps_init, \
                 tc.tile_pool(name="lnrow", bufs=1) as lnrow:
                for k in ["ln1g", "ln1b", "ln2g", "ln2b"]:
                    row = lnrow.tile([1, D], F32R, tag=f"{k}_row", name=f"{k}_row")
                    nc.sync.dma_start(row[:], ln_in[k][:].bitcast(F32R))
                    bc = const.tile([P, D], F32, tag=f"{k}_bc", name=f"{k}_bc")
                    for jf in range(2):
                        psb = ps_init.tile([P, 512], F32, tag="lnbc", name="lnbc")
                        nc.tensor.matmul(psb[:], ones1r[:], row[:, 512 * jf:512 * (jf + 1)])
                        nc.vector.tensor_copy(bc[:, 512 * jf:512 * (jf + 1)], psb[:])
                    lnbc[k] = bc

            wsb = {}
            for k in wcol_in:
                wt = const.tile([P, TT, N], F32, tag=f"w_{k}", name=f"w_{k}")
                nc.sync.dma_start(wt[:], wcol_in[k].rearrange("(tt p) n -> p tt n", p=P))
                wsb[k] = wt

            # ---------- DRAM bounce buffers for collectives ----------
            qk_b = dram.tile([NC * 2 * P, T], F8, tag="qk_b", name="qk_b")
            qk_o = dram.tile([NC * 2 * P, T], F8, tag="qk_o", name="qk_o")
            v_b = dram.tile([BS, P], F8, tag="v_b", name="v_b")
            v_o = dram.tile([BS, P], F8, tag="v_o", name="v_o")
            ab_b = dram.tile([NC * P, T], F8, tag="ab_b", name="ab_b")
            ab_o = dram.tile([NC * P, T], F8, tag="ab_o", name="ab_o")

            hpool = ctx.enter_context(tc.tile_pool(name="hpool", bufs=1))
            h_q = [hpool.tile([P, R], F32, tag=f"hq{tt}", name=f"hq{tt}") for tt in range(TT)]
            h_k = [hpool.tile([P, R], F32, tag=f"hk{tt}", name=f"hk{tt}") for tt in range(TT)]
            h_v = [hpool.tile([P, R], F32, tag=f"hv{tt}", name=f"hv{tt}") for tt in range(TT)]

            # ================= stage A: LN1 + transpose + features ========
            with tc.tile_pool(name="stagea", bufs=1) as stagea:
                nxT = stagea.tile([P, DCP, 2, T], F8, tag="nxT", name="nxT")
                with tc.tile_pool(name="ps_tr", bufs=2, space="PSUM") as ps_tr:
                    for tt in range(TT):
                        nx_t = stagea.tile([P, D], BF, tag="nxa", name=f"nxa{tt}", bufs=2)
                        _layernorm(nc, cpool, x_t[tt], nx_t,
                                   lnbc["ln1g"], lnbc["ln1b"], eps_t, f"ln1_{tt}")
                        for dcp in range(DCP):
                            pst = ps_tr.tile([P, 2, P], BF, tag="tp", name="tp")
                            for i in range(2):
                                dc = 2 * dcp + i
                                nc.tensor.transpose(pst[:, i], nx_t[:, P * dc:P * (dc + 1)],
                                                    identb[:])
                            nc.vector.tensor_copy(
                                nxT[:, dcp, :, P * tt:P * (tt + 1)], pst[:])

                with tc.tile_pool(name="ps_feat", bufs=8, space="PSUM") as ps_feat:
                    for tt in range(TT):
                        psf = [ps_feat.tile([P, 512], F32, tag="feat", name=f"feat{tt}_{g}")
                               for g in range(8)]
                        for dcp in range(DCP):
                            lhs = nxT[:, dcp, :, P * tt:P * (tt + 1)]
                            for g in range(8):
                                src = (fqk_sb if g < 4 else fv_sb)
                                gg = g % 4
                                nc.tensor.matmul(
                                    psf[g][:], lhs, src[:, dcp, :, 2 * gg:2 * (gg + 1), :],
                                    start=(dcp == 0), stop=(dcp == DCP - 1), perf_mode=DR)
                        for m in range(N):
                            bank, half = m // 2, m % 2
                            pq = psf[bank][:, R * half:R * (half + 1)]
                            pv = psf[4 + bank][:, R * half:R * (half + 1)]
                            for htiles, wk, ps_slice in (
                                    (h_q, "wfq", pq), (h_k, "wfk", pq), (h_v, "wfv", pv)):
                                wcol = wsb[wk][:, tt, m:m + 1]
                                if m == 0:
                                    nc.vector.tensor_scalar_mul(htiles[tt][:], ps_slice, wcol)
                                else:
                                    nc.vector.scalar_tensor_tensor(
                                        htiles[tt][:], ps_slice, wcol, htiles[tt][:],
                                        op0=OP.mult, op1=OP.add)

            if STAGES == 1:
                for tt in range(TT):
                    hq_ev = cpool.tile([P, R], F32, tag="hq_ev", name=f"hq_ev{tt}")
                    nc.vector.tensor_copy(hq_ev[:], h_q[tt][:])
                    nc.sync.dma_start(out_ap[P * tt:P * (tt + 1), 0:R], hq_ev[:])

            # ================= stage B: restores + A2A =====================
            if STAGES >= 2:
                with tc.tile_pool(name="rp", bufs=1) as rp, \
                     tc.tile_pool(name="bpool", bufs=1) as bpool, \
                     tc.tile_pool(name="ap_pool", bufs=2) as ap_pool, \
                     tc.tile_pool(name="ps_bt", bufs=2, space="PSUM") as ps_bt, \
                     tc.tile_pool(name="ps_bc", bufs=2, space="PSUM") as ps_bc, \
                     tc.tile_pool(name="ps_r", bufs=4, space="PSUM") as ps_r, \
                     tc.tile_pool(name="ev_b", bufs=3) as ev_b:
                    rqk_sb = rp.tile([P, N, RC, D], F8, tag="rqk_sb", name="rqk_sb")
                    rv_sb = rp.tile([P, N, RC, D], F8, tag="rv_sb", name="rv_sb")
                    for n in range(N):
                        nc.sync.dma_start(
                            rqk_sb[:, n], rqk_in[n].rearrange("(rc p) d -> p rc d", p=P))
                        nc.sync.dma_start(
                            rv_sb[:, n], rv_in[n].rearrange("(rc p) d -> p rc d", p=P))
                    # wbc rows: PE-broadcast of transposed routing weights
                    wbc = {}
                    for k in ["wrqT", "wrkT", "wrvT"]:
                        tiles = []
                        for n in range(N):
                            rowt = bpool.tile([1, T], BF, tag="wrow", name=f"{k}row{n}", bufs=2)
                            nc.sync.dma_start(rowt[:], wrow_in[k][n:n + 1, :])
                            psb = ps_bc.tile([P, T], F32, tag="wbc_ps", name="wbc_ps")
                            nc.tensor.matmul(psb[:], ones_bf[:], rowt[:])
                            wt = bpool.tile([P, T], BF, tag=f"wbc_{k}", name=f"wbc_{k}{n}")
                            nc.scalar.activation(wt[:], psb[:], AF.Copy)
                            tiles.append(wt)
                        wbc[k] = tiles
                    # bf16 casts of h + paired transposes (once per h)
                    hT = {}
                    for key, htiles in (("q", h_q), ("k", h_k), ("v", h_v)):
                        ht = bpool.tile([P, RC, T], BF, tag=f"hT{key}", name=f"hT{key}")
                        for tt in range(TT):
                            hbf = cpool.tile([P, R], BF, tag="hbf", name=f"hbf{key}{tt}")
                            nc.vector.tensor_copy(hbf[:], htiles[tt][:])
                            pst = ps_bt.tile([P, 2, P], BF, tag="tpb", name="tpb")
                            for rc in range(RC):
                                nc.tensor.transpose(pst[:, rc], hbf[:, P * rc:P * (rc + 1)],
                                                    identb[:])
                            nc.vector.tensor_copy(ht[:, :, P * tt:P * (tt + 1)], pst[:])
                        hT[key] = ht

                    def build_A(hkey, wkey):
                        A = [ap_pool.tile([P, RC, T], F8, tag=f"A{n}", name=f"A_{wkey}{n}")
                             for n in range(N)]
                        for n in range(N):
                            for rc in range(RC):
                                nc.vector.tensor_mul(
                                    A[n][:, rc, :], hT[hkey][:, rc, :], wbc[wkey][n][:])
                        return A

                    def qk_restore(A, row_off):
                        for dm in range(DC):
                            ps = ps_r.tile([P, T], F32, tag="r_ps", name="r_ps")
                            for n in range(N):
                                nc.tensor.matmul(
                                    ps[:], rqk_sb[:, n, :, P * dm:P * (dm + 1)], A[n][:],
                                    start=(n == 0), stop=(n == N - 1), perf_mode=DR)
                            ev = ev_b.tile([P, T], F8, tag="ev_qk", name="ev_qk")
                            nc.scalar.activation(ev[:], ps[:], AF.Copy)
                            nc.sync.dma_start(
                                qk_b[2 * P * dm + row_off: 2 * P * dm + row_off + P, :],
                                ev[:])

                    Aq = build_A("q", "wrqT")
                    qk_restore(Aq, 0)
                    Ak = build_A("k", "wrkT")
                    qk_restore(Ak, P)
                    nc.gpsimd.collective_compute(
                        "AllToAll", OP.bypass, replica_groups=[list(range(NC))],
                        ins=[qk_b.opt()], outs=[qk_o.opt()])
                    Av = build_A("v", "wrvT")
                    for tt in range(TT):
                        for jf in range(2):
                            ps = ps_r.tile([P, 512], F32, tag="r_ps", name="v_ps")
                            for n in range(N):
                                nc.tensor.matmul(
                                    ps[:], Av[n][:, :, P * tt:P * (tt + 1)],
                                    rv_sb[:, n, :, 512 * jf:512 * (jf + 1)],
                                    start=(n == 0), stop=(n == N - 1), perf_mode=DR)
                            ev = ev_b.tile([P, 512], F8, tag="ev_v", name="ev_v")
                            nc.vector.tensor_copy(ev[:], ps[:])
                            for db in range(4):
                                d = 4 * jf + db
                                nc.sync.dma_start(
                                    v_b[T * d + P * tt: T * d + P * (tt + 1), :],
                                    ev[:, P * db:P * (db + 1)])
                    nc.gpsimd.collective_compute(
                        "AllToAll", OP.bypass, replica_groups=[list(range(NC))],
                        ins=[v_b.opt()], outs=[v_o.opt()])

            if STAGES == 2:
                for tt in range(TT):
                    qo_ev = cpool.tile([P, T], F8, tag="qo_ev", name=f"qo_ev{tt}")
                    nc.sync.dma_start(qo_ev[:], qk_o[2 * P * tt:2 * P * tt + P, :])
                    qo_f = cpool.tile([P, T], F32, tag="qo_f", name=f"qo_f{tt}")
                    nc.vector.tensor_copy(qo_f[:], qo_ev[:])
                    nc.sync.dma_start(out_ap[P * tt:P * (tt + 1), 0:T], qo_f[:])

            # ================= attention (head-sharded, packed) ============
            if STAGES >= 3:
                with tc.tile_pool(name="qkv_bh", bufs=1) as qkv_bh, \
                     tc.tile_pool(name="pt_pool", bufs=12) as pt_pool, \
                     tc.tile_pool(name="ps_st", bufs=4, space="PSUM") as ps_st, \
                     tc.tile_pool(name="ps_o", bufs=4, space="PSUM") as ps_o:
                    qt2s, kt2s, vpss = [], [], []
                    for b in range(B):
                        qt2 = qkv_bh.tile([P, S], F8, tag=f"qt2_{b}", name=f"qt2_{b}")
                        kt2 = qkv_bh.tile([P, S], F8, tag=f"kt2_{b}", name=f"kt2_{b}")
                        for sl in range(4):
                            s = 4 * b + sl
                            nc.sync.dma_start(qt2[:, 512 * sl:512 * (sl + 1)],
                                              qk_o[2 * P * s:2 * P * s + P, :])
                            nc.sync.dma_start(kt2[:, 512 * sl:512 * (sl + 1)],
                                              qk_o[2 * P * s + P:2 * P * (s + 1), :])
                        vps = []
                        for h2 in range(2):
                            vp = qkv_bh.tile([P, S // P // 2, 2, 80], F8,
                                             tag=f"vp{h2}_{b}", name=f"vp{h2}_{b}")
                            nc.sync.dma_start(
                                vp[:, :, :, 0:DH],
                                v_o[S * b: S * (b + 1), DH * h2: DH * (h2 + 1)]
                                .rearrange("(u i p) f -> p u i f", p=P, i=2))
                            nc.vector.memset(vp[:, :, :, DH:DH + 1], 1.0)
                            vps.append(vp)
                        qt2s.append(qt2); kt2s.append(kt2); vpss.append(vps)

                    for b in range(B):
                        qt2, kt2, vps = qt2s[b], kt2s[b], vpss[b]
                        for qg in range(4):
                            o_ps = [ps_o.tile([DH + 1, 512], F32, tag="o_ps",
                                              name=f"o{b}_{qg}_{h2}") for h2 in range(2)]
                            nkt = 4 * qg + 4
                            for u in range(nkt // 2):
                                pt2 = [pt_pool.tile([P, 2, 512], F8, tag="pt",
                                                    name=f"pt{b}_{qg}_{u}_{h2}")
                                       for h2 in range(2)]
                                for i in range(2):
                                    kt = 2 * u + i
                                    j = kt - 4 * qg
                                    for h2 in range(2):
                                        st = ps_st.tile([P, 512], F32, tag="st", name="st")
                                        nc.tensor.matmul(
                                            st[:],
                                            kt2[DH * h2:DH * (h2 + 1), P * kt:P * (kt + 1)],
                                            qt2[DH * h2:DH * (h2 + 1),
                                                512 * qg:512 * (qg + 1)])
                                        pt = pt2[h2][:, i, :]
                                        if j < 0:
                                            nc.scalar.activation(pt, st[:], AF.Exp,
                                                                 scale=0.125)
                                        else:
                                            if j > 0:
                                                nc.vector.memset(pt[:, 0:P * j], 0.0)
                                            nc.scalar.activation(
                                                pt[:, P * j:], st[:, P * j:],
                                                AF.Exp, scale=0.125)
                                            nc.vector.tensor_mul(
                                                pt[:, P * j:P * (j + 1)],
                                                pt[:, P * j:P * (j + 1)], masku[:])
                                for h2 in range(2):
                                    nc.tensor.matmul(
                                        o_ps[h2][:], vps[h2][:, u, :, 0:DH + 1], pt2[h2][:],
                                        start=(u == 0), stop=(u == nkt // 2 - 1),
                                        perf_mode=DR)
                            for h2 in range(2):
                                den = cpool.tile([1, 512], F32, tag="den", name="den")
                                nc.vector.tensor_copy(den[:], o_ps[h2][DH:DH + 1, :])
                                rec = cpool.tile([1, 512], BF, tag="rec", name="rec")
                                with nc.allow_low_precision(reason="bf16 rsqrt broadcast"):
                                    nc.scalar.activation(rec[:], den[:],
                                                         AF.Abs_reciprocal_sqrt)
                                bc = ps_st.tile([DH, 512], F32, tag="st", name="bc")
                                nc.tensor.matmul(bc[:], ones_bf[:, 0:DH], rec[:])
                                bc_sb = cpool.tile([DH, 512], BF, tag="bc_sb", name="bc_sb")
                                nc.scalar.activation(bc_sb[:], bc[:], AF.Copy)
                                # o / den = o * rsqrt(den)^2
                                t1 = cpool.tile([DH, 512], BF, tag="t1", name="t1")
                                nc.vector.tensor_mul(t1[:], o_ps[h2][0:DH, :], bc_sb[:])
                                nrm = cpool.tile([DH, 512], F8, tag="nrm", name="nrm")
                                nc.vector.tensor_mul(nrm[:], t1[:], bc_sb[:])
                                sblk = 4 * b + qg
                                nc.sync.dma_start(
                                    ab_b[P * sblk + DH * h2: P * sblk + DH * (h2 + 1), :],
                                    nrm[:])
                nc.gpsimd.collective_compute(
                    "AllToAll", OP.bypass, replica_groups=[list(range(NC))],
                    ins=[ab_b.opt()], outs=[ab_o.opt()])

            if STAGES == 3:
                for tt in range(TT):
                    at_ev = cpool.tile([P, T], F8, tag="at_ev", name=f"at_ev{tt}")
                    nc.sync.dma_start(at_ev[:], ab_o[P * tt:P * (tt + 1), :])
                    at_f = cpool.tile([P, T], F32, tag="at_f", name=f"at_f{tt}")
                    nc.vector.tensor_copy(at_f[:], at_ev[:])
                    nc.sync.dma_start(out_ap[P * tt:P * (tt + 1), 0:T], at_f[:])

            # ============ W_O (token-local) + stage C ======================
            if STAGES >= 4:
                with tc.tile_pool(name="cpers", bufs=1) as cpers, \
                     tc.tile_pool(name="cscr", bufs=2) as cscr, \
                     tc.tile_pool(name="ps_wo", bufs=2, space="PSUM") as ps_wo, \
                     tc.tile_pool(name="ps_ct", bufs=2, space="PSUM") as ps_ct, \
                     tc.tile_pool(name="ps_kf", bufs=2, space="PSUM") as ps_kf, \
                     tc.tile_pool(name="ps_kr", bufs=2, space="PSUM") as ps_kr:
                    wo_sb = cpers.tile([P, DCP, 2, D], F8, tag="wo_sb", name="wo_sb")
                    aT = cpers.tile([P, DCP, 2, T], F8, tag="aT", name="aT")
                    for dcp in range(DCP):
                        for i in range(2):
                            dc = 2 * dcp + i
                            nc.sync.dma_start(wo_sb[:, dcp, i],
                                              wo_in[P * dc:P * (dc + 1), :])
                            nc.sync.dma_start(aT[:, dcp, i],
                                              ab_o[P * dc:P * (dc + 1), :])
                    x2 = [cpers.tile([P, D], F32, tag=f"x2_{tt}", name=f"x2_{tt}")
                          for tt in range(TT)]
                    nx2T = cpers.tile([P, DCP, 2, T], F8, tag="nx2T", name="nx2T")
                    for tt in range(TT):
                        for jf in range(2):
                            ps = ps_wo.tile([P, 512], F32, tag="wo_ps", name="wo_ps")
                            for dcp in range(DCP):
                                nc.tensor.matmul(
                                    ps[:], aT[:, dcp, :, P * tt:P * (tt + 1)],
                                    wo_sb[:, dcp, :, 512 * jf:512 * (jf + 1)],
                                    start=(dcp == 0), stop=(dcp == DCP - 1), perf_mode=DR)
                            nc.vector.tensor_add(
                                x2[tt][:, 512 * jf:512 * (jf + 1)],
                                x_t[tt][:, 512 * jf:512 * (jf + 1)], ps[:])
                        if STAGES >= 5:
                            nx2 = cscr.tile([P, D], BF, tag="nx2", name=f"nx2_{tt}")
                            _layernorm(nc, cpool, x2[tt], nx2,
                                       lnbc["ln2g"], lnbc["ln2b"], eps_t, f"ln2_{tt}")
                            for dcp in range(DCP):
                                pst = ps_ct.tile([P, 2, P], BF, tag="tpc", name="tpc")
                                for i in range(2):
                                    dc = 2 * dcp + i
                                    nc.tensor.transpose(
                                        pst[:, i], nx2[:, P * dc:P * (dc + 1)], identb[:])
                                nc.vector.tensor_copy(
                                    nx2T[:, dcp, :, P * tt:P * (tt + 1)], pst[:])

                    if STAGES == 4:
                        for tt in range(TT):
                            ao_ev = cpool.tile([P, D], F32, tag="ao_ev", name=f"ao_ev{tt}")
                            nc.vector.tensor_copy(ao_ev[:], x2[tt][:])
                            nc.sync.dma_start(out_ap[P * tt:P * (tt + 1), :], ao_ev[:])

                    if STAGES >= 5:
                        fkn_sb = cpers.tile([P, DCP, 2, N, KR], F8, tag="fkn_sb", name="fkn_sb")
                        rkn_sb = cpers.tile([P, N, D], F8, tag="rkn_sb", name="rkn_sb")
                        for dcp in range(DCP):
                            for i in range(2):
                                dc = 2 * dcp + i
                                nc.sync.dma_start(
                                    fkn_sb[:, dcp, i],
                                    fkn_in[:, P * dc:P * (dc + 1), :].rearrange("n p f -> p n f"))
                        for n in range(N):
                            nc.sync.dma_start(rkn_sb[:, n], rkn_in[n])
                        wbc_kr = []
                        for n in range(N):
                            rowt = cscr.tile([1, T], BF, tag="krrow", name=f"krrow{n}")
                            nc.sync.dma_start(rowt[:], wrow_in["wkrT"][n:n + 1, :])
                            psb = ps_kr.tile([P, T], F32, tag="kr_ps", name="wbckr_ps")
                            nc.tensor.matmul(psb[:], ones_bf[:], rowt[:])
                            wt = cpers.tile([P, T], BF, tag=f"wbc_kr{n}", name=f"wbc_kr{n}")
                            nc.scalar.activation(wt[:], psb[:], AF.Copy)
                            wbc_kr.append(wt)

                        hknT = cpers.tile([P, T], BF, tag="hknT", name="hknT")
                        for tt in range(TT):
                            psk = [ps_kf.tile([P, 512], F32, tag="kf", name=f"kf{tt}_{g}")
                                   for g in range(2)]
                            for dcp in range(DCP):
                                lhs = nx2T[:, dcp, :, P * tt:P * (tt + 1)]
                                for g in range(2):
                                    nc.tensor.matmul(
                                        psk[g][:], lhs, fkn_sb[:, dcp, :, 4 * g:4 * (g + 1), :],
                                        start=(dcp == 0), stop=(dcp == DCP - 1), perf_mode=DR)
                            hkn = cscr.tile([P, KR], F32, tag="hkn", name=f"hkn{tt}")
                            for m in range(N):
                                pslice = psk[m // 4][:, KR * (m % 4):KR * (m % 4 + 1)]
                                wcol = wsb["wkf"][:, tt, m:m + 1]
                                if m == 0:
                                    nc.vector.tensor_scalar_mul(hkn[:], pslice, wcol)
                                else:
                                    nc.vector.scalar_tensor_tensor(
                                        hkn[:], pslice, wcol, hkn[:],
                                        op0=OP.mult, op1=OP.add)
                            hknb = cscr.tile([P, KR], BF, tag="hknb", name=f"hknb{tt}")
                            nc.vector.tensor_copy(hknb[:], hkn[:])
                            pst = ps_ct.tile([P, P], BF, tag="tpc", name="tpc_kn")
                            nc.tensor.transpose(pst[:], hknb[:], identb[:])
                            nc.vector.tensor_copy(hknT[:, P * tt:P * (tt + 1)], pst[:])

                        Akn = cpers.tile([P, N, T], F8, tag="Akn", name="Akn")
                        for n in range(N):
                            nc.vector.tensor_mul(Akn[:, n, :], hknT[:], wbc_kr[n][:])
                        for tt in range(TT):
                            for jf in range(2):
                                ps = ps_kr.tile([P, 512], F32, tag="kr_ps", name="kn_ps")
                                for u in range(N // 2):
                                    nc.tensor.matmul(
                                        ps[:], Akn[:, 2 * u:2 * (u + 1), P * tt:P * (tt + 1)],
                                        rkn_sb[:, 2 * u:2 * (u + 1), 512 * jf:512 * (jf + 1)],
                                        start=(u == 0), stop=(u == N // 2 - 1), perf_mode=DR)
                                out_sb = cscr.tile([P, 512], F32, tag="out_sb", name="out_sb")
                                nc.vector.tensor_add(
                                    out_sb[:], x2[tt][:, 512 * jf:512 * (jf + 1)], ps[:])
                                nc.sync.dma_start(
                                    out_ap[P * tt:P * (tt + 1), 512 * jf:512 * (jf + 1)],
                                    out_sb[:])

    nc.compile()
    return nc


_NC = None


def _get_nc():
    global _NC
    if _NC is None:
        _NC = _build()
    return _NC


def prepare_in_maps(inputs):
    bf = ml_dtypes.bfloat16
    f8 = ml_dtypes.float8_e4m3
    inp = {k: np.ascontiguousarray(np.asarray(v, dtype=np.float32)) for k, v in inputs.items()}
    x_flat = inp["x"].reshape(BS, D)
    wcols = {
        "wfq": inp["fqk_w_Q"].reshape(BS, N), "wfk": inp["fqk_w_K"].reshape(BS, N),
        "wfv": inp["fv_w"].reshape(BS, N), "wkf": inp["feature_know_w"].reshape(BS, N),
    }
    wrows = {
        "wrqT": inp["rqk_w_Q"].reshape(BS, N), "wrkT": inp["rqk_w_K"].reshape(BS, N),
        "wrvT": inp["rv_w"].reshape(BS, N), "wkrT": inp["restore_know_w"].reshape(BS, N),
    }
    pools = {
        "fqk_p": inp["f_qk"].astype(f8), "fv_p": inp["f_v"].astype(f8),
        "rqk_p": inp["r_qk"].astype(f8), "rv_p": inp["r_v"].astype(f8),
        "fkn_p": inp["f_know"].astype(f8), "rkn_p": inp["r_know"].astype(f8),
    }
    wo_p = np.ascontiguousarray(inp["W_O"].T).astype(f8)
    masku = np.ascontiguousarray(np.tril(np.ones((P, P), np.float32)).T).astype(f8)

    in_maps = []
    for c in range(NC):
        sl = slice(T * c, T * (c + 1))
        m = {
            "x_sh": np.ascontiguousarray(x_flat[sl]),
            "wo_p": wo_p, "masku": masku,
            "ln1g": inp["ln1_g"].reshape(1, D), "ln1b": inp["ln1_b"].reshape(1, D),
            "ln2g": inp["ln2_g"].reshape(1, D), "ln2b": inp["ln2_b"].reshape(1, D),
        }
        m.update(pools)
        for k, v in wcols.items():
            m[k] = np.ascontiguousarray(v[sl])
        for k, v in wrows.items():
            m[k] = np.ascontiguousarray(v[sl].T).astype(bf)
        in_maps.append(m)
    return in_maps


def kernel(**inputs):
    nc = _get_nc()
    in_maps = prepare_in_maps(inputs)
    res = run_bass_kernel_spmd(nc, in_maps, list(range(NC))).results
    out = np.concatenate([res[c]["out_sh"] for c in range(NC)], axis=0)
    return out.reshape(B, S, D)


# revision 9
# speedup vs baseline: 1.9384x; 1.0158x over previous
"""Trainium2 Bass kernel for nn_DAWNBlock (8-core SPMD), v3.

Decomposition (validated in numpy: proto_check.py, quant_check.py):
  - Token-sharded: core c owns flat tokens [512c, 512c+512) of [B*S=4096].
    LN1, feature einsums, restores (Q/K/V), LN2 + knowledge run token-local.
  - Head-sharded attention: core c owns d-cols [128c, 128c+128) (= heads
    {2c, 2c+1}). One AllToAll reshards Q^T+K^T together (1MB fp8), a second
    reshards V (0.5MB fp8); attention output is AllToAll'd back to token
    shards (0.5MB fp8) and W_O applied token-locally.
  - fp8e4m3 + DoubleRow (K=256 pairs, 2x PE rate) for features, restores,
    PV, W_O and knowledge matmuls; scores fp8 at K=64 packed as concurrent
    row-group pairs (both heads). Quantization study: all-fp8 rel err 4e-3
    vs the 2e-2 gate. Accumulation fp32 in PSUM; LN stats + residual fp32.
  - Routing weights: feature PSUM banks combined via DVE scalar_tensor_tensor;
    restores use A[n] = hT * wbc[n] (wbc = PE-broadcast row of host-
    pretransposed weights), h transposed once.
  - Causal softmax without max-subtraction; denominator via ones-column in V
    (PSUM row 64); reciprocal via ACT rsqrt + squared broadcast multiply.
"""
import sys

sys.path.insert(0, "/opt/trn_rl_repo")

import os
import numpy as np
import ml_dtypes
import concourse.bass as bass
import concourse.mybir as mybir
import concourse.tile as tile
from concourse import bacc
from concourse.bass_utils import run_bass_kernel_spmd
from concourse.masks import make_identity

B, S, D, H, R, N, KR = 2, 2048, 1024, 16, 256, 8, 128
DH = D // H           # 64
BS = B * S            # 4096
NC = 8
T = BS // NC          # 512 tokens per core
P = 128
TT = T // P           # 4 token tiles per core
DC = D // P           # 8 d chunks
DCP = DC // 2         # 4 d chunk-pairs (DoubleRow)
RC = R // P           # 2 r chunks
EPS = 1e-5

STAGES = int(os.environ.get("BASS_STAGES", "5"))
F32 = mybir.dt.float32
F32R = mybir.dt.float32r
BF = mybir.dt.bfloat16
F8 = mybir.dt.float8e4
DR = mybir.MatmulPerfMode.DoubleRow
AF = mybir.ActivationFunctionType
OP = mybir.AluOpType


def _layernorm(nc, cpool, x_sb, nx_sb, eps_tile, tag):
    """nx = (x - mean(x)) * rsqrt(var + eps) for one [128, D] tile
    (LN gain is folded into the downstream pools host-side; bias is zero)."""
    s = cpool.tile([P, 1], F32, tag="ln_s", name=f"{tag}_s")
    nm = cpool.tile([P, 1], F32, tag="ln_nm", name=f"{tag}_nm")
    sq = cpool.tile([P, D], F32, tag="ln_sq", name=f"{tag}_sq")
    ssq = cpool.tile([P, 1], F32, tag="ln_ssq", name=f"{tag}_ssq")
    rs = cpool.tile([P, 1], F32, tag="ln_rs", name=f"{tag}_rs")
    nmrs = cpool.tile([P, 1], F32, tag="ln_nmrs", name=f"{tag}_nmrs")
    nc.vector.reduce_sum(s[:], x_sb[:], axis=mybir.AxisListType.X)
    nc.vector.tensor_scalar_mul(nm[:], s[:], -1.0 / D)
    nc.scalar.activation(sq[:], x_sb[:], AF.Square, bias=nm[:], accum_out=ssq[:])
    nc.scalar.activation(rs[:], ssq[:], AF.Abs_reciprocal_sqrt,
                         bias=eps_tile[:], scale=1.0 / D)
    nc.vector.tensor_mul(nmrs[:], nm[:], rs[:])
    nc.scalar.activation(nx_sb[:], x_sb[:], AF.Identity, bias=nmrs[:], scale=rs[:])


def _build():
    nc = bacc.Bacc("TRN2", target_bir_lowering=False, debug=False, num_devices=NC)

    def di(name, shape, dt=F8):
        return nc.dram_tensor(name, shape, dt, kind="ExternalInput").ap()

    x_in = di("x_sh", [T, D], F32)
    wcol_in = {k: di(k, [T, N], F32) for k in ["wfq", "wfk", "wfv", "wkf"]}
    wrow_in = {k: di(k, [N, T], BF) for k in ["wrqT", "wrkT", "wrvT", "wkrT"]}
    fqk_in = di("fqk_p", [N, D, R])
    fv_in = di("fv_p", [N, D, R])
    rqk_in = di("rqk_p", [N, R, D])
    rv_in = di("rv_p", [N, R, D])
    fkn_in = di("fkn_p", [N, D, KR])
    rkn_in = di("rkn_p", [N, KR, D])
    wo_in = di("wo_p", [D, D])       # = W_O.T
    masku_in = di("masku", [P, P])
    out_ap = nc.dram_tensor("out_sh", [T, D], F32, kind="ExternalOutput").ap()

    with tile.TileContext(nc) as tc:
        from contextlib import ExitStack
        with ExitStack() as ctx:
            const = ctx.enter_context(tc.tile_pool(name="const", bufs=1))
            cpool = ctx.enter_context(tc.tile_pool(name="scratch", bufs=2))
            dram = ctx.enter_context(tc.tile_pool(name="dram", bufs=1, space="DRAM"))

            # ---------- persistent pools / big prefetches first ----------
            xpool = ctx.enter_context(tc.tile_pool(name="xpool", bufs=1))
            x_t = [xpool.tile([P, D], F32, tag=f"x{tt}", name=f"x{tt}") for tt in range(TT)]
            for tt in range(TT):
                nc.sync.dma_start(x_t[tt][:], x_in[P * tt:P * (tt + 1), :])

            fwpool = ctx.enter_context(tc.tile_pool(name="fwpool", bufs=1))
            fqk_sb = fwpool.tile([P, DCP, 2, N, R], F8, tag="fqk_sb", name="fqk_sb")
            fv_sb = fwpool.tile([P, DCP, 2, N, R], F8, tag="fv_sb", name="fv_sb")
            for dcp in range(DCP):
                for i in range(2):
                    dc = 2 * dcp + i
                    nc.sync.dma_start(
                        fqk_sb[:, dcp, i],
                        fqk_in[:, P * dc:P * (dc + 1), :].rearrange("n p r -> p n r"))
                    nc.sync.dma_start(
                        fv_sb[:, dcp, i],
                        fv_in[:, P * dc:P * (dc + 1), :].rearrange("n p r -> p n r"))

            rp = ctx.enter_context(tc.tile_pool(name="rp", bufs=1))
            rqk_sb = rp.tile([P, N, RC, D], F8, tag="rqk_sb", name="rqk_sb")
            rv_sb = rp.tile([P, N, RC, D], F8, tag="rv_sb", name="rv_sb")
            for n in range(N):
                nc.sync.dma_start(
                    rqk_sb[:, n], rqk_in[n].rearrange("(rc p) d -> p rc d", p=P))
                nc.sync.dma_start(
                    rv_sb[:, n], rv_in[n].rearrange("(rc p) d -> p rc d", p=P))
            # tiny warm-up collective: pays the first-collective ncfw latency
            # (~10us) here, overlapped with stage A, instead of on A2A(qk)
            dumm_b = dram.tile([NC * 8, 8], F32, tag="dumm_b", name="dumm_b")
            dumm_o = dram.tile([NC * 8, 8], F32, tag="dumm_o", name="dumm_o")
            nc.gpsimd.collective_compute(
                "AllToAll", OP.bypass, replica_groups=[list(range(NC))],
                ins=[dumm_b.opt()], outs=[dumm_o.opt()])

            ident_f = const.tile([P, P], F32, tag="ident_f", name="ident_f")
            make_identity(nc, ident_f)
            ident8 = const.tile([P, P], F8, tag="ident8", name="ident8")
            nc.vector.tensor_copy(ident8[:], ident_f[:])
            identb = const.tile([P, P], BF, tag="identb", name="identb")
            nc.vector.tensor_copy(identb[:], ident_f[:])
            ones_bf = const.tile([1, P], BF, tag="ones_bf", name="ones_bf")
            nc.vector.memset(ones_bf[:], 1.0)
            masku = const.tile([P, P], F8, tag="masku", name="masku")
            nc.sync.dma_start(masku[:], masku_in[:])
            eps_t = const.tile([P, 1], F32, tag="eps", name="eps")
            nc.vector.memset(eps_t[:], EPS)

            wsb = {}
            for k in wcol_in:
                wt = const.tile([P, TT, N], F32, tag=f"w_{k}", name=f"w_{k}")
                nc.sync.dma_start(wt[:], wcol_in[k].rearrange("(tt p) n -> p tt n", p=P))
                wsb[k] = wt

            # ---------- DRAM bounce buffers for collectives ----------
            qk_b = dram.tile([NC * 2 * P, T], F8, tag="qk_b", name="qk_b")
            qk_o = dram.tile([NC * 2 * P, T], F8, tag="qk_o", name="qk_o")
            v_b = dram.tile([BS, P], F8, tag="v_b", name="v_b")
            v_o = dram.tile([BS, P], F8, tag="v_o", name="v_o")
            ab_b = dram.tile([NC * P, T], F8, tag="ab_b", name="ab_b")
            ab_o = dram.tile([NC * P, T], F8, tag="ab_o", name="ab_o")

            cprep = ctx.enter_context(tc.tile_pool(name="cprep", bufs=1))
            wo_sb = cprep.tile([P, DCP, 2, D], F8, tag="wo_sb", name="wo_sb")
            fkn_sb = cprep.tile([P, DCP, 2, N, KR], F8, tag="fkn_sb", name="fkn_sb")
            rkn_sb = cprep.tile([P, N, D], F8, tag="rkn_sb", name="rkn_sb")
            wbc_kr = []

            hpool = ctx.enter_context(tc.tile_pool(name="hpool", bufs=1))
            h_q = [hpool.tile([P, R], F32, tag=f"hq{tt}", name=f"hq{tt}") for tt in range(TT)]
            h_k = [hpool.tile([P, R], F32, tag=f"hk{tt}", name=f"hk{tt}") for tt in range(TT)]
            h_v = [hpool.tile([P, R], F32, tag=f"hv{tt}", name=f"hv{tt}") for tt in range(TT)]

            # ================= stage A: LN1 + transpose + features ========
            with tc.tile_pool(name="stagea", bufs=1) as stagea:
                nxT = stagea.tile([P, DCP, 2, T], F8, tag="nxT", name="nxT")
                with tc.tile_pool(name="ps_tr", bufs=2, space="PSUM") as ps_tr:
                    for tt in range(TT):
                        nx_t = stagea.tile([P, D], BF, tag="nxa", name=f"nxa{tt}", bufs=2)
                        _layernorm(nc, cpool, x_t[tt], nx_t, eps_t, f"ln1_{tt}")
                        for dcp in range(DCP):
                            pst = ps_tr.tile([P, 2, P], BF, tag="tp", name="tp")
                            for i in range(2):
                                dc = 2 * dcp + i
                                nc.tensor.transpose(pst[:, i], nx_t[:, P * dc:P * (dc + 1)],
                                                    identb[:])
                            nc.vector.tensor_copy(
                                nxT[:, dcp, :, P * tt:P * (tt + 1)], pst[:])

                with tc.tile_pool(name="ps_feat", bufs=8, space="PSUM") as ps_feat:
                    for tt in range(TT):
                        psf = [ps_feat.tile([P, 512], F32, tag="feat", name=f"feat{tt}_{g}")
                               for g in range(8)]
                        for dcp in range(DCP):
                            lhs = nxT[:, dcp, :, P * tt:P * (tt + 1)]
                            for g in range(8):
                                src = (fqk_sb if g < 4 else fv_sb)
                                gg = g % 4
                                nc.tensor.matmul(
                                    psf[g][:], lhs, src[:, dcp, :, 2 * gg:2 * (gg + 1), :],
                                    start=(dcp == 0), stop=(dcp == DCP - 1), perf_mode=DR)
                        for m in range(N):
                            bank, half = m // 2, m % 2
                            pq = psf[bank][:, R * half:R * (half + 1)]
                            pv = psf[4 + bank][:, R * half:R * (half + 1)]
                            for htiles, wk, ps_slice in (
                                    (h_q, "wfq", pq), (h_k, "wfk", pq), (h_v, "wfv", pv)):
                                wcol = wsb[wk][:, tt, m:m + 1]
                                if m == 0:
                                    nc.vector.tensor_scalar_mul(htiles[tt][:], ps_slice, wcol)
                                else:
                                    nc.vector.scalar_tensor_tensor(
                                        htiles[tt][:], ps_slice, wcol, htiles[tt][:],
                                        op0=OP.mult, op1=OP.add)

            if STAGES == 1:
                for tt in range(TT):
                    hq_ev = cpool.tile([P, R], F32, tag="hq_ev", name=f"hq_ev{tt}")
                    nc.vector.tensor_copy(hq_ev[:], h_q[tt][:])
                    nc.sync.dma_start(out_ap[P * tt:P * (tt + 1), 0:R], hq_ev[:])

            # ================= stage B: restores + A2A =====================
            if STAGES >= 2:
                with tc.tile_pool(name="bpool", bufs=1) as bpool, \
                     tc.tile_pool(name="ap_pool", bufs=2) as ap_pool, \
                     tc.tile_pool(name="ps_bt", bufs=2, space="PSUM") as ps_bt, \
                     tc.tile_pool(name="ps_bc", bufs=2, space="PSUM") as ps_bc, \
                     tc.tile_pool(name="ps_r", bufs=4, space="PSUM") as ps_r, \
                     tc.tile_pool(name="ev_b", bufs=3) as ev_b:
                    # wbc rows: PE-broadcast of transposed routing weights
                    wbc = {}
                    for k in ["wrqT", "wrkT", "wrvT"]:
                        tiles = []
                        for n in range(N):
                            rowt = bpool.tile([1, T], BF, tag="wrow", name=f"{k}row{n}", bufs=2)
                            nc.sync.dma_start(rowt[:], wrow_in[k][n:n + 1, :])
                            psb = ps_bc.tile([P, T], F32, tag="wbc_ps", name="wbc_ps")
                            nc.tensor.matmul(psb[:], ones_bf[:], rowt[:])
                            wt = bpool.tile([P, T], BF, tag=f"wbc_{k}", name=f"wbc_{k}{n}")
                            nc.scalar.activation(wt[:], psb[:], AF.Copy)
                            tiles.append(wt)
                        wbc[k] = tiles
                    # bf16 casts of h + paired transposes (once per h)
                    hT = {}
                    for key, htiles in (("q", h_q), ("k", h_k), ("v", h_v)):
                        ht = bpool.tile([P, RC, T], BF, tag=f"hT{key}", name=f"hT{key}")
                        for tt in range(TT):
                            hbf = cpool.tile([P, R], BF, tag="hbf", name=f"hbf{key}{tt}")
                            nc.vector.tensor_copy(hbf[:], htiles[tt][:])
                            pst = ps_bt.tile([P, 2, P], BF, tag="tpb", name="tpb")
                            for rc in range(RC):
                                nc.tensor.transpose(pst[:, rc], hbf[:, P * rc:P * (rc + 1)],
                                                    identb[:])
                            nc.vector.tensor_copy(ht[:, :, P * tt:P * (tt + 1)], pst[:])
                        hT[key] = ht

                    def build_A(hkey, wkey):
                        A = [ap_pool.tile([P, RC, T], F8, tag=f"A{n}", name=f"A_{wkey}{n}")
                             for n in range(N)]
                        for n in range(N):
                            for rc in range(RC):
                                nc.vector.tensor_mul(
                                    A[n][:, rc, :], hT[hkey][:, rc, :], wbc[wkey][n][:])
                        return A

                    def qk_restore(A, row_off):
                        for dm in range(DC):
                            ps = ps_r.tile([P, T], F32, tag="r_ps", name="r_ps")
                            for n in range(N):
                                nc.tensor.matmul(
                                    ps[:], rqk_sb[:, n, :, P * dm:P * (dm + 1)], A[n][:],
                                    start=(n == 0), stop=(n == N - 1), perf_mode=DR)
                            ev = ev_b.tile([P, T], F8, tag="ev_qk", name="ev_qk")
                            nc.scalar.activation(ev[:], ps[:], AF.Copy)
                            nc.sync.dma_start(
                                qk_b[2 * P * dm + row_off: 2 * P * dm + row_off + P, :],
                                ev[:])

                    Aq = build_A("q", "wrqT")
                    qk_restore(Aq, 0)
                    Ak = build_A("k", "wrkT")
                    qk_restore(Ak, P)
                    nc.gpsimd.collective_compute(
                        "AllToAll", OP.bypass, replica_groups=[list(range(NC))],
                        ins=[qk_b.opt()], outs=[qk_o.opt()])
                    Av = build_A("v", "wrvT")
                    for tt in range(TT):
                        for jf in range(2):
                            ps = ps_r.tile([P, 512], F32, tag="r_ps", name="v_ps")
                            for n in range(N):
                                nc.tensor.matmul(
                                    ps[:], Av[n][:, :, P * tt:P * (tt + 1)],
                                    rv_sb[:, n, :, 512 * jf:512 * (jf + 1)],
                                    start=(n == 0), stop=(n == N - 1), perf_mode=DR)
                            ev = ev_b.tile([P, 512], F8, tag="ev_v", name="ev_v")
                            nc.vector.tensor_copy(ev[:], ps[:])
                            for db in range(4):
                                d = 4 * jf + db
                                nc.sync.dma_start(
                                    v_b[T * d + P * tt: T * d + P * (tt + 1), :],
                                    ev[:, P * db:P * (db + 1)])
                    nc.gpsimd.collective_compute(
                        "AllToAll", OP.bypass, replica_groups=[list(range(NC))],
                        ins=[v_b.opt()], outs=[v_o.opt()])
                    # prefetch stage-C weights + build wbc_kr during the A2A window
                    for dcp in range(DCP):
                        for i in range(2):
                            dc = 2 * dcp + i
                            nc.sync.dma_start(wo_sb[:, dcp, i],
                                              wo_in[P * dc:P * (dc + 1), :])
                            nc.sync.dma_start(
                                fkn_sb[:, dcp, i],
                                fkn_in[:, P * dc:P * (dc + 1), :].rearrange("n p f -> p n f"))
                    for n in range(N):
                        nc.sync.dma_start(rkn_sb[:, n], rkn_in[n])
                    for n in range(N):
                        rowt = bpool.tile([1, T], BF, tag="wrow", name=f"krrow{n}", bufs=2)
                        nc.sync.dma_start(rowt[:], wrow_in["wkrT"][n:n + 1, :])
                        psb = ps_bc.tile([P, T], F32, tag="wbc_ps", name="wbckr_ps")
                        nc.tensor.matmul(psb[:], ones_bf[:], rowt[:])
                        wt = cprep.tile([P, T], BF, tag=f"wbc_kr{n}", name=f"wbc_kr{n}")
                        nc.scalar.activation(wt[:], psb[:], AF.Copy)
                        wbc_kr.append(wt)

            if STAGES == 2:
                for tt in range(TT):
                    qo_ev = cpool.tile([P, T], F8, tag="qo_ev", name=f"qo_ev{tt}")
                    nc.sync.dma_start(qo_ev[:], qk_o[2 * P * tt:2 * P * tt + P, :])
                    qo_f = cpool.tile([P, T], F32, tag="qo_f", name=f"qo_f{tt}")
                    nc.vector.tensor_copy(qo_f[:], qo_ev[:])
                    nc.sync.dma_start(out_ap[P * tt:P * (tt + 1), 0:T], qo_f[:])

            # ================= attention (head-sharded, packed) ============
            if STAGES >= 3:
                with tc.tile_pool(name="qkv_bh", bufs=1) as qkv_bh, \
                     tc.tile_pool(name="pt_pool", bufs=20) as pt_pool, \
                     tc.tile_pool(name="ps_st", bufs=4, space="PSUM") as ps_st, \
                     tc.tile_pool(name="ps_o", bufs=4, space="PSUM") as ps_o:
                    qt2s, kt2s, vpss = [], [], []
                    for b in range(B):
                        qt2 = qkv_bh.tile([P, S], F8, tag=f"qt2_{b}", name=f"qt2_{b}")
                        kt2 = qkv_bh.tile([P, S], F8, tag=f"kt2_{b}", name=f"kt2_{b}")
                        for sl in range(4):
                            s = 4 * b + sl
                            nc.sync.dma_start(qt2[:, 512 * sl:512 * (sl + 1)],
                                              qk_o[2 * P * s:2 * P * s + P, :])
                            nc.sync.dma_start(kt2[:, 512 * sl:512 * (sl + 1)],
                                              qk_o[2 * P * s + P:2 * P * (s + 1), :])
                        vps = []
                        for h2 in range(2):
                            vp = qkv_bh.tile([P, S // P // 2, 2, 80], F8,
                                             tag=f"vp{h2}_{b}", name=f"vp{h2}_{b}")
                            nc.sync.dma_start(
                                vp[:, :, :, 0:DH],
                                v_o[S * b: S * (b + 1), DH * h2: DH * (h2 + 1)]
                                .rearrange("(u i p) f -> p u i f", p=P, i=2))
                            nc.vector.memset(vp[:, :, :, DH:DH + 1], 1.0)
                            vps.append(vp)
                        qt2s.append(qt2); kt2s.append(kt2); vpss.append(vps)

                    for b in range(B):
                        qt2, kt2, vps = qt2s[b], kt2s[b], vpss[b]
                        for qg in range(4):
                            o_ps = [ps_o.tile([DH + 1, 512], F32, tag="o_ps",
                                              name=f"o{b}_{qg}_{h2}") for h2 in range(2)]
                            nkt = 4 * qg + 4
                            for u in range(nkt // 2):
                                pt2 = [pt_pool.tile([P, 2, 512], F8, tag="pt",
                                                    name=f"pt{b}_{qg}_{u}_{h2}")
                                       for h2 in range(2)]
                                for i in range(2):
                                    kt = 2 * u + i
                                    j = kt - 4 * qg
                                    for h2 in range(2):
                                        st = ps_st.tile([P, 512], F32, tag="st", name="st")
                                        nc.tensor.matmul(
                                            st[:],
                                            kt2[DH * h2:DH * (h2 + 1), P * kt:P * (kt + 1)],
                                            qt2[DH * h2:DH * (h2 + 1),
                                                512 * qg:512 * (qg + 1)])
                                        pt = pt2[h2][:, i, :]
                                        if j < 0:
                                            nc.scalar.activation(pt, st[:], AF.Exp,
                                                                 scale=0.125)
                                        else:
                                            if j > 0:
                                                nc.vector.memset(pt[:, 0:P * j], 0.0)
                                            nc.scalar.activation(
                                                pt[:, P * j:], st[:, P * j:],
                                                AF.Exp, scale=0.125)
                                            nc.vector.tensor_mul(
                                                pt[:, P * j:P * (j + 1)],
                                                pt[:, P * j:P * (j + 1)], masku[:])
                                for h2 in range(2):
                                    nc.tensor.matmul(
                                        o_ps[h2][:], vps[h2][:, u, :, 0:DH + 1], pt2[h2][:],
                                        start=(u == 0), stop=(u == nkt // 2 - 1),
                                        perf_mode=DR)
                            for h2 in range(2):
                                den = cpool.tile([1, 512], F32, tag="den", name="den")
                                nc.vector.tensor_copy(den[:], o_ps[h2][DH:DH + 1, :])
                                rec = cpool.tile([1, 512], BF, tag="rec", name="rec")
                                with nc.allow_low_precision(reason="bf16 reciprocal broadcast"):
                                    nc.vector.reciprocal(rec[:], den[:])
                                bc = ps_st.tile([DH, 512], F32, tag="st", name="bc")
                                nc.tensor.matmul(bc[:], ones_bf[:, 0:DH], rec[:])
                                bc_sb = cpool.tile([DH, 512], BF, tag="bc_sb", name="bc_sb")
                                nc.vector.tensor_copy(bc_sb[:], bc[:])
                                nrm = cpool.tile([DH, 512], F8, tag="nrm", name="nrm")
                                nc.vector.tensor_mul(nrm[:], o_ps[h2][0:DH, :], bc_sb[:])
                                sblk = 4 * b + qg
                                nc.sync.dma_start(
                                    ab_b[P * sblk + DH * h2: P * sblk + DH * (h2 + 1), :],
                                    nrm[:])
                nc.gpsimd.collective_compute(
                    "AllToAll", OP.bypass, replica_groups=[list(range(NC))],
                    ins=[ab_b.opt()], outs=[ab_o.opt()])

            if STAGES == 3:
                for tt in range(TT):
                    at_ev = cpool.tile([P, T], F8, tag="at_ev", name=f"at_ev{tt}")
                    nc.sync.dma_start(at_ev[:], ab_o[P * tt:P * (tt + 1), :])
                    at_f = cpool.tile([P, T], F32, tag="at_f", name=f"at_f{tt}")
                    nc.vector.tensor_copy(at_f[:], at_ev[:])
                    nc.sync.dma_start(out_ap[P * tt:P * (tt + 1), 0:T], at_f[:])

            # ============ W_O (token-local) + stage C ======================
            if STAGES >= 4:
                with tc.tile_pool(name="cpers", bufs=1) as cpers, \
                     tc.tile_pool(name="cscr", bufs=2) as cscr, \
                     tc.tile_pool(name="ps_acc", bufs=4, space="PSUM") as ps_acc, \
                     tc.tile_pool(name="ps_ct", bufs=2, space="PSUM") as ps_ct, \
                     tc.tile_pool(name="ps_kr", bufs=2, space="PSUM") as ps_kr:
                    aT = cpers.tile([P, DCP, 2, T], F8, tag="aT", name="aT")
                    for dcp in range(DCP):
                        for i in range(2):
                            dc = 2 * dcp + i
                            nc.sync.dma_start(aT[:, dcp, i],
                                              ab_o[P * dc:P * (dc + 1), :])
                    x2 = [cpers.tile([P, D], F32, tag=f"x2_{tt}", name=f"x2_{tt}")
                          for tt in range(TT)]
                    nx2T = cpers.tile([P, DCP, 2, T], F8, tag="nx2T", name="nx2T")
                    for tt in range(TT):
                        for jf in range(2):
                            ps = ps_acc.tile([P, 512], F32, tag="acc", name="wo_ps")
                            for dcp in range(DCP):
                                nc.tensor.matmul(
                                    ps[:], aT[:, dcp, :, P * tt:P * (tt + 1)],
                                    wo_sb[:, dcp, :, 512 * jf:512 * (jf + 1)],
                                    start=(dcp == 0), stop=(dcp == DCP - 1), perf_mode=DR)
                            nc.vector.tensor_add(
                                x2[tt][:, 512 * jf:512 * (jf + 1)],
                                x_t[tt][:, 512 * jf:512 * (jf + 1)], ps[:])
                        if STAGES >= 5:
                            nx2 = cscr.tile([P, D], BF, tag="nx2", name=f"nx2_{tt}")
                            _layernorm(nc, cpool, x2[tt], nx2, eps_t, f"ln2_{tt}")
                            for dcp in range(DCP):
                                pst = ps_ct.tile([P, 2, P], BF, tag="tpc", name="tpc")
                                for i in range(2):
                                    dc = 2 * dcp + i
                                    nc.tensor.transpose(
                                        pst[:, i], nx2[:, P * dc:P * (dc + 1)], identb[:])
                                nc.vector.tensor_copy(
                                    nx2T[:, dcp, :, P * tt:P * (tt + 1)], pst[:])

                    if STAGES == 4:
                        for tt in range(TT):
                            ao_ev = cpool.tile([P, D], F32, tag="ao_ev", name=f"ao_ev{tt}")
                            nc.vector.tensor_copy(ao_ev[:], x2[tt][:])
                            nc.sync.dma_start(out_ap[P * tt:P * (tt + 1), :], ao_ev[:])

                    if STAGES >= 5:
                        hknT = cpers.tile([P, T], BF, tag="hknT", name="hknT")
                        for tt in range(TT):
                            psk = [ps_acc.tile([P, 512], F32, tag="acc", name=f"kf{tt}_{g}")
                                   for g in range(2)]
                            for dcp in range(DCP):
                                lhs = nx2T[:, dcp, :, P * tt:P * (tt + 1)]
                                for g in range(2):
                                    nc.tensor.matmul(
                                        psk[g][:], lhs, fkn_sb[:, dcp, :, 4 * g:4 * (g + 1), :],
                                        start=(dcp == 0), stop=(dcp == DCP - 1), perf_mode=DR)
                            hkn = cscr.tile([P, KR], F32, tag="hkn", name=f"hkn{tt}")
                            for m in range(N):
                                pslice = psk[m // 4][:, KR * (m % 4):KR * (m % 4 + 1)]
                                wcol = wsb["wkf"][:, tt, m:m + 1]
                                if m == 0:
                                    nc.vector.tensor_scalar_mul(hkn[:], pslice, wcol)
                                else:
                                    nc.vector.scalar_tensor_tensor(
                                        hkn[:], pslice, wcol, hkn[:],
                                        op0=OP.mult, op1=OP.add)
                            hknb = cscr.tile([P, KR], BF, tag="hknb", name=f"hknb{tt}")
                            nc.vector.tensor_copy(hknb[:], hkn[:])
                            pst = ps_ct.tile([P, P], BF, tag="tpc", name="tpc_kn")
                            nc.tensor.transpose(pst[:], hknb[:], identb[:])
                            nc.vector.tensor_copy(hknT[:, P * tt:P * (tt + 1)], pst[:])

                        Akn = cpers.tile([P, N, T], F8, tag="Akn", name="Akn")
                        for n in range(N):
                            nc.vector.tensor_mul(Akn[:, n, :], hknT[:], wbc_kr[n][:])
                        for tt in range(TT):
                            for jf in range(2):
                                ps = ps_kr.tile([P, 512], F32, tag="kr_ps", name="kn_ps")
                                for u in range(N // 2):
                                    nc.tensor.matmul(
                                        ps[:], Akn[:, 2 * u:2 * (u + 1), P * tt:P * (tt + 1)],
                                        rkn_sb[:, 2 * u:2 * (u + 1), 512 * jf:512 * (jf + 1)],
                                        start=(u == 0), stop=(u == N // 2 - 1), perf_mode=DR)
                                out_sb = cscr.tile([P, 512], F32, tag="out_sb", name="out_sb")
                                nc.vector.tensor_add(
                                    out_sb[:], x2[tt][:, 512 * jf:512 * (jf + 1)], ps[:])
                                nc.sync.dma_start(
                                    out_ap[P * tt:P * (tt + 1), 512 * jf:512 * (jf + 1)],
                                    out_sb[:])

    nc.compile()
    return nc


_NC = None


def _get_nc():
    global _NC
    if _NC is None:
        _NC = _build()
    return _NC


def prepare_in_maps(inputs):
    bf = ml_dtypes.bfloat16
    f8 = ml_dtypes.float8_e4m3
    inp = {k: np.ascontiguousarray(np.asarray(v, dtype=np.float32)) for k, v in inputs.items()}
    x_flat = inp["x"].reshape(BS, D)
    wcols = {
        "wfq": inp["fqk_w_Q"].reshape(BS, N), "wfk": inp["fqk_w_K"].reshape(BS, N),
        "wfv": inp["fv_w"].reshape(BS, N), "wkf": inp["feature_know_w"].reshape(BS, N),
    }
    wrows = {
        "wrqT": inp["rqk_w_Q"].reshape(BS, N), "wrkT": inp["rqk_w_K"].reshape(BS, N),
        "wrvT": inp["rv_w"].reshape(BS, N), "wkrT": inp["restore_know_w"].reshape(BS, N),
    }
    g1 = inp["ln1_g"][None, :, None]
    g2 = inp["ln2_g"][None, :, None]
    assert np.abs(inp["ln1_b"]).max() == 0 and np.abs(inp["ln2_b"]).max() == 0, \
        "nonzero LN bias not supported by this build"
    pools = {
        "fqk_p": (inp["f_qk"] * g1).astype(f8), "fv_p": (inp["f_v"] * g1).astype(f8),
        "rqk_p": inp["r_qk"].astype(f8), "rv_p": inp["r_v"].astype(f8),
        "fkn_p": (inp["f_know"] * g2).astype(f8), "rkn_p": inp["r_know"].astype(f8),
    }
    wo_p = np.ascontiguousarray(inp["W_O"].T).astype(f8)
    masku = np.ascontiguousarray(np.tril(np.ones((P, P), np.float32)).T).astype(f8)

    in_maps = []
    for c in range(NC):
        sl = slice(T * c, T * (c + 1))
        m = {
            "x_sh": np.ascontiguousarray(x_flat[sl]),
            "wo_p": wo_p, "masku": masku,
        }
        m.update(pools)
        for k, v in wcols.items():
            m[k] = np.ascontiguousarray(v[sl])
        for k, v in wrows.items():
            m[k] = np.ascontiguousarray(v[sl].T).astype(bf)
        in_maps.append(m)
    return in_maps


def kernel(**inputs):
    nc = _get_nc()
    in_maps = prepare_in_maps(inputs)
    res = run_bass_kernel_spmd(nc, in_maps, list(range(NC))).results
    out = np.concatenate([res[c]["out_sh"] for c in range(NC)], axis=0)
    return out.reshape(B, S, D)
